# revision 1
# baseline (speedup 1.0000x reference)
"""Trainium2 Bass kernel for nn_EncoderText (4-layer SRU text encoder).

Reference computation:
  e = embed[x]                       # [B, T, K]
  4x SRU layers over time (layer0: k=4 projections incl highway; 1-3: k=3)
  gather last valid timestep per sequence, L2-normalize over features.

Strategy:
- Data-parallel over sequences across 8 NeuronCores, no collectives.
- Only VALID tokens are computed: sequences are LPT-bin-packed into
  NCH chunks of CCH columns per core (all cores share one program
  shape; the assignment of sequences to cores/chunks is host-side).
- Activations are channel-major [D, tokens] so the SRU recurrence
  c_t = f_t*c_{t-1} + (1-f_t)*cand_t maps onto the DVE
  tensor_tensor_scan along the free axis; a boundary mask (runtime
  input) zeroes f at each sequence start so sequences chain safely
  through one scan per chunk.
- Matmuls run in float32r (FP22, full PE rate at N>=256).
- Last-timestep selection: (iota == selcol) * h with accum_out, one
  fused DVE op per (channel-tile, slot); slot->sequence mapping is
  undone on the host.
"""

from contextlib import ExitStack

import numpy as np

import concourse.bass as bass
import concourse.mybir as mybir
import concourse.tile as tile
from concourse import bacc
from concourse.bass_utils import run_bass_kernel_spmd
from concourse.masks import make_identity

FP32 = mybir.dt.float32
F32R = mybir.dt.float32r
I32 = mybir.dt.int32
AF = mybir.ActivationFunctionType
OP = mybir.AluOpType

P = 128
N_CORES = 8
W_CONTIG = True  # host-pre-tiled contiguous weight DMA vs strided from raw W


def _ptiles(n):
    """[(start, size)] partition tiles of <=128 covering n."""
    out = []
    s = 0
    while s < n:
        out.append((s, min(P, n - s)))
        s += P
    return out


# ---------------------------------------------------------------------------
# Host-side packing plan
# ---------------------------------------------------------------------------


class Plan:
    def __init__(self, NCH, CCH, SMAX, bins):
        self.NCH = NCH  # chunks per core
        self.CCH = CCH  # columns per chunk
        self.SMAX = SMAX  # max sequences per chunk
        self.bins = bins  # [N_CORES][NCH] -> list of global seq ids
        self.NTOT = NCH * CCH
        self.NSLOT = NCH * SMAX


def make_plan(lengths):
    """LPT bin-pack sequences into N_CORES*NCH bins of capacity CCH."""
    lengths = np.asarray(lengths, np.int64)
    B = len(lengths)
    order = np.argsort(-lengths, kind="stable")
    total = int(lengths.sum())
    maxlen = int(lengths.max())

    best = None
    for NCH in range(1, 17):
        nbins = N_CORES * NCH
        loads = np.zeros(nbins, np.int64)
        bins = [[] for _ in range(nbins)]
        for i in order:
            j = int(np.argmin(loads))
            loads[j] += int(lengths[i])
            bins[j].append(int(i))
        C = int(loads.max())
        CCH = max(256, maxlen, -(-C // 8) * 8)
        if CCH > 480:
            continue
        cost = NCH * CCH
        if best is None or cost < best[0]:
            SMAX = max(len(b) for b in bins)
            best = (cost, NCH, CCH, SMAX, bins)
        if NCH * 256 >= best[0]:
            break
    assert best is not None, "no feasible packing"
    _, NCH, CCH, SMAX, bins = best
    core_bins = [bins[c * NCH : (c + 1) * NCH] for c in range(N_CORES)]
    plan = Plan(NCH, CCH, SMAX, core_bins)
    assert plan.NSLOT <= P, f"too many slots {plan.NSLOT}"
    return plan


# ---------------------------------------------------------------------------
# Device program
# ---------------------------------------------------------------------------


def build_program(K, D, V, NCH, CCH, SMAX):
    """Emit the per-core program.

    DRAM parameters (per core):
      xidx   [NTOKP]      int32  packed token row indices (pad -> 0)
      embed  [V, K]       f32    full embedding table (replicated)
      W0 [K, 4D], W1..W3 [D, 3D] f32r
      bf{l}, br{l} [128, NKD] f32  per-channel-tile bias columns
      bmask  [128, NTOT]  f32    0 at each sequence start (and padding), else 1
      selcol [128, NSLOT] f32    in-chunk column of slot's last token, -1 if none
      iotac  [128, CCH]   f32    0..CCH-1 per row
      out    [NSLOT, D]   f32    normalized last-step hidden per slot
    """
    NTOT = NCH * CCH
    NSLOT = NCH * SMAX
    NTOKP = ((NTOT + P - 1) // P) * P
    NJ = NTOKP // P
    KT_IN = _ptiles(K)
    KT_D = _ptiles(D)
    NKI, NKD = len(KT_IN), len(KT_D)
    TOK_T = _ptiles(NTOT)
    PSW = max(CCH, P)

    nc = bacc.Bacc("TRN2", target_bir_lowering=False, debug=False)

    xidx = nc.declare_dram_parameter("xidx", [NTOKP], I32, isOutput=False)
    emb = nc.declare_dram_parameter("embed", [V, K], FP32, isOutput=False)
    # weights: either host-pre-tiled (contiguous per-partition DMA) or raw
    Wd = []
    for l in range(4):
        gates_l = 4 if l == 0 else 3
        nki_l = len(KT_IN) if l == 0 else len(KT_D)
        if W_CONTIG:
            Wd.append(
                nc.declare_dram_parameter(
                    f"W{l}t", [NKD * gates_l, P, nki_l * P], F32R, isOutput=False
                )
            )
        else:
            Wd.append(
                nc.declare_dram_parameter(
                    f"W{l}t",
                    [K if l == 0 else D, gates_l * D],
                    F32R,
                    isOutput=False,
                )
            )
    bfd = [
        nc.declare_dram_parameter(f"bf{l}", [P, NKD], FP32, isOutput=False)
        for l in range(4)
    ]
    brd = [
        nc.declare_dram_parameter(f"br{l}", [P, NKD], FP32, isOutput=False)
        for l in range(4)
    ]
    bmask_d = nc.declare_dram_parameter("bmask", [P, NTOT], FP32, isOutput=False)
    selcol_d = nc.declare_dram_parameter("selcol", [P, NSLOT], FP32, isOutput=False)
    iotac_d = nc.declare_dram_parameter("iotac", [P, CCH], FP32, isOutput=False)
    out_d = nc.declare_dram_parameter("out", [NSLOT, D], FP32, isOutput=True)

    with tile.TileContext(nc) as tc, ExitStack() as ctx:
        sb = ctx.enter_context(tc.tile_pool(name="sb", bufs=1))
        big = ctx.enter_context(tc.tile_pool(name="big", bufs=NKI + 2 * NKD))
        wp = ctx.enter_context(tc.tile_pool(name="wp", bufs=1))
        tp = ctx.enter_context(tc.tile_pool(name="tp", bufs=2))
        pp = ctx.enter_context(tc.tile_pool(name="pp", bufs=8, space="PSUM"))

        # ---- constants ----
        identity = sb.tile([P, P], FP32, tag="identity")
        make_identity(nc, identity[:])
        bmask = sb.tile([P, NTOT], FP32, tag="bmask")
        nc.sync.dma_start(out=bmask[:], in_=bmask_d[:, :])
        selcol = sb.tile([P, NSLOT], FP32, tag="selcol")
        nc.sync.dma_start(out=selcol[:], in_=selcol_d[:, :])
        iotac = sb.tile([P, CCH], FP32, tag="iotac")
        nc.sync.dma_start(out=iotac[:], in_=iotac_d[:, :])
        bfs, brs = [], []
        for l in range(4):
            bft = sb.tile([P, NKD], FP32, tag=f"bf{l}")
            nc.sync.dma_start(out=bft[:], in_=bfd[l][:, :])
            bfs.append(bft)
            brt = sb.tile([P, NKD], FP32, tag=f"br{l}")
            nc.sync.dma_start(out=brt[:], in_=brd[l][:, :])
            brs.append(brt)
        idx_sb = sb.tile([P, NJ], I32, tag="idx")
        nc.sync.dma_start(
            out=idx_sb[:], in_=xidx[:].rearrange("(j p) -> p j", p=P)
        )

        # ---- embedding gather + transpose to channel-major e_T ----
        eT = []
        for k in range(NKI):
            et = big.tile([P, NTOT], F32R, tag="hb", name=f"eT{k}")
            eT.append(et)
        for j, (ts_, tj) in enumerate(TOK_T):
            eg = sb.tile([P, K], FP32, tag="eg", bufs=2, name=f"eg{j}")
            nc.gpsimd.indirect_dma_start(
                out=eg[:tj, :],
                out_offset=None,
                in_=emb[:, :],
                in_offset=bass.IndirectOffsetOnAxis(
                    ap=idx_sb[:tj, j : j + 1], axis=0
                ),
            )
            for k, (ks, kk) in enumerate(KT_IN):
                pt = pp.tile([P, PSW], FP32, tag="pt", name=f"ptr{j}_{k}")
                nc.tensor.transpose(
                    out=pt[:kk, :tj],
                    in_=eg[:tj, ks : ks + kk],
                    identity=identity[:tj, :tj],
                )
                nc.vector.tensor_copy(
                    out=eT[k][:kk, ts_ : ts_ + tj], in_=pt[:kk, :tj]
                )

        # ---- SRU layers ----
        hsel = []
        for ci in range(NKD):
            hs = sb.tile([P, NSLOT], FP32, tag="hsel", bufs=NKD, name=f"hsel{ci}")
            hsel.append(hs)

        in_tiles = eT
        ktin = KT_IN
        for l in range(4):
            gates = 4 if l == 0 else 3
            Kin = K if l == 0 else D
            nki = len(ktin)
            main_rows = (nki - 1) * P
            rem = Kin - main_rows

            new_h = None
            if l < 3:
                new_h = [
                    big.tile([P, NTOT], F32R, tag="hb", name=f"h{l}_{ci}")
                    for ci in range(NKD)
                ]

            for ci, (cs, mc) in enumerate(KT_D):
                wts = []
                for g in range(gates):
                    wt = wp.tile(
                        [P, nki, P],
                        F32R,
                        tag=f"w{g}",
                        bufs=2,
                        name=f"w{l}_{ci}_{g}",
                    )
                    if W_CONTIG:
                        nc.sync.dma_start(
                            out=wt[:, :, :],
                            in_=Wd[l][ci * gates + g, :, :].rearrange(
                                "p (kt m) -> p kt m", m=P
                            ),
                        )
                    else:
                        col0 = g * D + cs
                        if main_rows > 0:
                            nc.sync.dma_start(
                                out=wt[:, : nki - 1, :mc],
                                in_=Wd[l][0:main_rows, col0 : col0 + mc].rearrange(
                                    "(kt p) n -> p kt n", p=P
                                ),
                            )
                        nc.sync.dma_start(
                            out=wt[:rem, nki - 1, :mc],
                            in_=Wd[l][main_rows:Kin, col0 : col0 + mc],
                        )
                    wts.append(wt)

                for n in range(NCH):
                    nsl = slice(n * CCH, (n + 1) * CCH)
                    ps = []
                    for g in range(gates):
                        pt = pp.tile(
                            [P, PSW], FP32, tag="pt", name=f"pm{l}_{ci}_{n}_{g}"
                        )
                        for k2, (ks2, kk2) in enumerate(ktin):
                            nc.tensor.matmul(
                                out=pt[:mc, :CCH],
                                lhsT=wts[g][:kk2, k2, :mc],
                                rhs=in_tiles[k2][:kk2, nsl],
                                start=(k2 == 0),
                                stop=(k2 == nki - 1),
                            )
                        ps.append(pt)
                    cand = ps[0][:mc, :CCH]
                    fpre = ps[1][:mc, :CCH]
                    rpre = ps[2][:mc, :CCH]

                    fsb = tp.tile([P, CCH], FP32, tag="fsb", name=f"f{l}_{ci}_{n}")
                    nc.scalar.activation(
                        out=fsb[:mc, :],
                        in_=fpre,
                        func=AF.Sigmoid,
                        bias=bfs[l][:mc, ci : ci + 1],
                    )
                    rsb = tp.tile([P, CCH], FP32, tag="rsb", name=f"r{l}_{ci}_{n}")
                    nc.scalar.activation(
                        out=rsb[:mc, :],
                        in_=rpre,
                        func=AF.Sigmoid,
                        bias=brs[l][:mc, ci : ci + 1],
                    )
                    # z' = (f - 1) * cand  == -(1-f)*cand
                    zb = tp.tile([P, CCH], FP32, tag="zb", name=f"z{l}_{ci}_{n}")
                    nc.vector.scalar_tensor_tensor(
                        out=zb[:mc, :],
                        in0=fsb[:mc, :],
                        scalar=1.0,
                        in1=cand,
                        op0=OP.subtract,
                        op1=OP.mult,
                    )
                    # f masked at sequence starts (in place)
                    nc.vector.tensor_mul(
                        out=fsb[:mc, :], in0=fsb[:mc, :], in1=bmask[:mc, nsl]
                    )
                    # c_t = fm*c_{t-1} - z'
                    cst = tp.tile([P, CCH], FP32, tag="cst", name=f"c{l}_{ci}_{n}")
                    nc.vector.tensor_tensor_scan(
                        out=cst[:mc, :],
                        data0=fsb[:mc, :],
                        data1=zb[:mc, :],
                        initial=0.0,
                        op0=OP.mult,
                        op1=OP.subtract,
                    )
                    # h = r*tanh(c) + (1-r)*xres, built in place over cst
                    nc.scalar.activation(
                        out=cst[:mc, :], in_=cst[:mc, :], func=AF.Tanh
                    )
                    if l == 0:
                        xres = ps[3][:mc, :CCH]
                    else:
                        xres = in_tiles[ci][:mc, nsl]
                    nc.vector.tensor_sub(out=cst[:mc, :], in0=cst[:mc, :], in1=xres)
                    nc.vector.tensor_mul(
                        out=cst[:mc, :], in0=cst[:mc, :], in1=rsb[:mc, :]
                    )
                    if l < 3:
                        nc.vector.tensor_add(
                            out=new_h[ci][:mc, nsl], in0=cst[:mc, :], in1=xres
                        )
                    else:
                        hh = tp.tile([P, CCH], FP32, tag="hh", name=f"hh{l}_{ci}_{n}")
                        nc.vector.tensor_add(out=hh[:mc, :], in0=cst[:mc, :], in1=xres)
                        # select last-token column per slot:
                        # (iota == selcol[slot]) * h, summed along free
                        for kslot in range(SMAX):
                            slot = n * SMAX + kslot
                            scr = tp.tile(
                                [P, CCH], FP32, tag="scr", name=f"sc{ci}_{n}_{kslot}"
                            )
                            nc.vector.scalar_tensor_tensor(
                                out=scr[:mc, :],
                                in0=iotac[:mc, :],
                                scalar=selcol[:mc, slot : slot + 1],
                                in1=hh[:mc, :],
                                op0=OP.is_equal,
                                op1=OP.mult,
                                accum_out=hsel[ci][:mc, slot : slot + 1],
                            )

            if l < 3:
                in_tiles = new_h
                ktin = KT_D

        # ---- epilogue: transpose selected hidden, L2-normalize, write out ----
        out_sb = sb.tile([NSLOT, D], FP32, tag="out_sb")
        for ci, (cs, mc) in enumerate(KT_D):
            pt = pp.tile([P, PSW], FP32, tag="pt", name=f"pte{ci}")
            nc.tensor.transpose(
                out=pt[:NSLOT, :mc],
                in_=hsel[ci][:mc, :NSLOT],
                identity=identity[:mc, :mc],
            )
            nc.vector.tensor_copy(out=out_sb[:, cs : cs + mc], in_=pt[:NSLOT, :mc])

        # sum of squares, chunked through one PSUM bank
        nq = (D + CCH - 1) // CCH
        ssp = sb.tile([NSLOT, nq], FP32, tag="ssp")
        for q in range(nq):
            q0 = q * CCH
            qw = min(CCH, D - q0)
            pt = pp.tile([P, PSW], FP32, tag="pt", name=f"ptq{q}")
            nc.scalar.activation(
                out=pt[:NSLOT, :qw],
                in_=out_sb[:, q0 : q0 + qw],
                func=AF.Square,
                accum_out=ssp[:, q : q + 1],
            )
        ss = sb.tile([NSLOT, 1], FP32, tag="ss")
        nc.vector.tensor_reduce(
            out=ss[:], in_=ssp[:], axis=mybir.AxisListType.X, op=OP.add
        )
        # 1/sqrt(ss + eps); eps guards empty slots (zero rows)
        eps = sb.tile([NSLOT, 1], FP32, tag="eps")
        nc.gpsimd.memset(eps[:], 1e-20)
        sq = sb.tile([NSLOT, 1], FP32, tag="sq")
        nc.scalar.activation(out=sq[:], in_=ss[:], func=AF.Sqrt, bias=eps[:, 0:1])
        inv = sb.tile([NSLOT, 1], FP32, tag="inv")
        nc.vector.reciprocal(out=inv[:], in_=sq[:])
        nc.vector.tensor_scalar(
            out=out_sb[:], in0=out_sb[:], scalar1=inv[:, 0:1], scalar2=None, op0=OP.mult
        )
        nc.sync.dma_start(out=out_d[:, :], in_=out_sb[:])

    nc.compile()
    return nc


# ---------------------------------------------------------------------------
# Host-side input prep
# ---------------------------------------------------------------------------


def _retile_W(W, Kin, D, gates, NKD, NKI_l):
    """[Kin, gates*D] -> [NKD*gates, 128, nki*128] per-partition-contiguous."""
    nki = NKI_l
    Wp = np.zeros((nki * P, gates * NKD * P), np.float32)
    Dp = NKD * P
    src = np.asarray(W, np.float32)
    for g in range(gates):
        Wp[:Kin, g * Dp : g * Dp + D] = src[:, g * D : (g + 1) * D]
    # [kt*P+p, g*Dp + ci*P + m] -> [ci*gates+g, p, kt*P+m]
    Wp = Wp.reshape(nki, P, gates, NKD, P)
    Wt = np.ascontiguousarray(np.transpose(Wp, (3, 2, 1, 0, 4)))
    return Wt.reshape(NKD * gates, P, nki * P)


def _pack_bias(b_half, D, NKD):
    """[D] -> [128, NKD]: column ci holds channels ci*128..ci*128+127."""
    pad = NKD * P - D
    bp = np.pad(np.asarray(b_half, np.float32), (0, pad))
    return np.ascontiguousarray(bp.reshape(NKD, P).T)


def make_core_inputs(core, plan, x, lengths, embed, Ws, bs, K, D, V):
    NCH, CCH, SMAX = plan.NCH, plan.CCH, plan.SMAX
    NTOT, NSLOT = plan.NTOT, plan.NSLOT
    NTOKP = ((NTOT + P - 1) // P) * P
    NKD = len(_ptiles(D))

    xl = np.zeros(NTOKP, np.int32)
    bmask = np.zeros((1, NTOT), np.float32)
    selcol = np.full((1, NSLOT), -1.0, np.float32)
    for ch, bin_seqs in enumerate(plan.bins[core]):
        pos = 0
        for k, b in enumerate(bin_seqs):
            ln = int(lengths[b])
            if ln <= 0:
                continue
            col0 = ch * CCH + pos
            xl[col0 : col0 + ln] = x[b, :ln]
            bmask[0, col0 + 1 : col0 + ln] = 1.0
            selcol[0, ch * SMAX + k] = float(pos + ln - 1)
            pos += ln

    iotac = np.arange(CCH, dtype=np.float32)[None, :]

    im = {
        "xidx": xl,
        "embed": np.asarray(embed, np.float32),
        "bmask": np.broadcast_to(bmask, (P, NTOT)).copy(),
        "selcol": np.broadcast_to(selcol, (P, NSLOT)).copy(),
        "iotac": np.broadcast_to(iotac, (P, CCH)).copy(),
    }
    for l in range(4):
        im[f"W{l}t"] = Ws[l]
        im[f"bf{l}"] = _pack_bias(bs[l][:D], D, NKD)
        im[f"br{l}"] = _pack_bias(bs[l][D:], D, NKD)
    return im


_NC_CACHE = {}


def kernel(x, lengths, embed, W0, b0, W1, b1, W2, b2, W3, b3):
    x = np.asarray(x)
    lengths = np.asarray(lengths)
    embed = np.asarray(embed, np.float32)
    Ws = [np.asarray(w, np.float32) for w in (W0, W1, W2, W3)]
    bs = [np.asarray(b, np.float32) for b in (b0, b1, b2, b3)]

    Bb, T = x.shape
    V, K = embed.shape
    D = Ws[1].shape[0]

    plan = make_plan(lengths)
    key = (K, D, V, plan.NCH, plan.CCH, plan.SMAX)
    if key not in _NC_CACHE:
        _NC_CACHE[key] = build_program(*key)
    nc = _NC_CACHE[key]

    NKD = len(_ptiles(D))
    NKI = len(_ptiles(K))
    Wt = [
        _retile_W(Ws[0], K, D, 4, NKD, NKI),
        _retile_W(Ws[1], D, D, 3, NKD, NKD),
        _retile_W(Ws[2], D, D, 3, NKD, NKD),
        _retile_W(Ws[3], D, D, 3, NKD, NKD),
    ]
    in_maps = [
        make_core_inputs(c, plan, x, lengths, embed, Wt, bs, K, D, V)
        for c in range(N_CORES)
    ]
    res = run_bass_kernel_spmd(nc, in_maps, core_ids=list(range(N_CORES)))

    out = np.zeros((Bb, D), np.float32)
    for c in range(N_CORES):
        oc = res.results[c]["out"]
        for ch, bin_seqs in enumerate(plan.bins[c]):
            for k, b in enumerate(bin_seqs):
                out[b] = oc[ch * plan.SMAX + k]
    return out



# revision 3
# speedup vs baseline: 2.4081x; 2.4081x over previous
"""Trainium2 Bass kernel for nn_EncoderText (4-layer SRU text encoder).

Reference computation:
  e = embed[x]                       # [B, T, K]
  4x SRU layers over time (layer0: k=4 projections incl highway; 1-3: k=3)
  gather last valid timestep per sequence, L2-normalize over features.

Strategy:
- Data-parallel over sequences across 8 NeuronCores, no collectives.
- Only VALID tokens are computed: sequences are LPT-bin-packed into
  NCH chunks of CCH columns per core (all cores share one program
  shape; the assignment of sequences to cores/chunks is host-side).
- Activations are channel-major [D, tokens] so the SRU recurrence
  c_t = f_t*c_{t-1} + (1-f_t)*cand_t maps onto the DVE
  tensor_tensor_scan along the free axis; a boundary mask (runtime
  input) zeroes f at each sequence start so sequences chain safely
  through one scan per chunk.
- Matmuls run in float32r (FP22, full PE rate at N>=256).
- Last-timestep selection: (iota == selcol) * h with accum_out, one
  fused DVE op per (channel-tile, slot); slot->sequence mapping is
  undone on the host.
"""

from contextlib import ExitStack

import numpy as np

import concourse.bass as bass
import concourse.mybir as mybir
import concourse.tile as tile
from concourse import bacc
from concourse.bass_utils import run_bass_kernel_spmd
from concourse.masks import make_identity

FP32 = mybir.dt.float32
F32R = mybir.dt.float32r
BF16 = mybir.dt.bfloat16
I32 = mybir.dt.int32
AF = mybir.ActivationFunctionType
OP = mybir.AluOpType

P = 128
N_CORES = 8
W_CONTIG = True  # host-pre-tiled contiguous weight DMA vs strided from raw W


def _ptiles(n):
    """[(start, size)] partition tiles of <=128 covering n."""
    out = []
    s = 0
    while s < n:
        out.append((s, min(P, n - s)))
        s += P
    return out


# ---------------------------------------------------------------------------
# Host-side packing plan
# ---------------------------------------------------------------------------


class Plan:
    def __init__(self, NCH, CCH, SMAX, bins):
        self.NCH = NCH  # chunks per core
        self.CCH = CCH  # columns per chunk
        self.SMAX = SMAX  # max sequences per chunk
        self.bins = bins  # [N_CORES][NCH] -> list of global seq ids
        self.NTOT = NCH * CCH
        self.NSLOT = NCH * SMAX


def make_plan(lengths):
    """LPT bin-pack sequences into N_CORES*NCH bins of capacity CCH."""
    lengths = np.asarray(lengths, np.int64)
    B = len(lengths)
    order = np.argsort(-lengths, kind="stable")
    total = int(lengths.sum())
    maxlen = int(lengths.max())

    best = None
    for NCH in range(1, 17):
        nbins = N_CORES * NCH
        loads = np.zeros(nbins, np.int64)
        bins = [[] for _ in range(nbins)]
        for i in order:
            j = int(np.argmin(loads))
            loads[j] += int(lengths[i])
            bins[j].append(int(i))
        C = int(loads.max())
        CCH = max(256, maxlen, -(-C // 8) * 8)
        if CCH > 480:
            continue
        cost = NCH * CCH
        if best is None or cost < best[0]:
            SMAX = max(len(b) for b in bins)
            best = (cost, NCH, CCH, SMAX, bins)
        if NCH * 256 >= best[0]:
            break
    assert best is not None, "no feasible packing"
    _, NCH, CCH, SMAX, bins = best
    core_bins = [bins[c * NCH : (c + 1) * NCH] for c in range(N_CORES)]
    plan = Plan(NCH, CCH, SMAX, core_bins)
    assert plan.NSLOT <= P, f"too many slots {plan.NSLOT}"
    return plan


# ---------------------------------------------------------------------------
# Device program
# ---------------------------------------------------------------------------


def build_program(K, D, V, NCH, CCH, SMAX):
    """Emit the per-core program.

    DRAM parameters (per core):
      xidx   [NTOKP]      int32  packed token row indices (pad -> 0)
      embed  [V, K]       f32    full embedding table (replicated)
      W0 [K, 4D], W1..W3 [D, 3D] f32r
      bf{l}, br{l} [128, NKD] f32  per-channel-tile bias columns
      bmask  [128, NTOT]  f32    0 at each sequence start (and padding), else 1
      selcol [128, NSLOT] f32    in-chunk column of slot's last token, -1 if none
      iotac  [128, CCH]   f32    0..CCH-1 per row
      out    [NSLOT, D]   f32    normalized last-step hidden per slot
    """
    NTOT = NCH * CCH
    NSLOT = NCH * SMAX
    NTOKP = ((NTOT + P - 1) // P) * P
    NJ = NTOKP // P
    KT_IN = _ptiles(K)
    KT_D = _ptiles(D)
    NKI, NKD = len(KT_IN), len(KT_D)
    TOK_T = _ptiles(NTOT)
    PSW = max(CCH, P)

    nc = bacc.Bacc("TRN2", target_bir_lowering=False, debug=False)

    xidx = nc.declare_dram_parameter("xidx", [NTOKP], I32, isOutput=False)
    emb = nc.declare_dram_parameter("embed", [V, K], FP32, isOutput=False)
    # weights: either host-pre-tiled (contiguous per-partition DMA) or raw
    Wd = []
    for l in range(4):
        gates_l = 4 if l == 0 else 3
        nki_l = len(KT_IN) if l == 0 else len(KT_D)
        if W_CONTIG:
            Wd.append(
                nc.declare_dram_parameter(
                    f"W{l}t", [NKD * gates_l, P, nki_l * P], BF16, isOutput=False
                )
            )
        else:
            Wd.append(
                nc.declare_dram_parameter(
                    f"W{l}t",
                    [K if l == 0 else D, gates_l * D],
                    F32R,
                    isOutput=False,
                )
            )
    bfd = [
        nc.declare_dram_parameter(f"bf{l}", [P, NKD], FP32, isOutput=False)
        for l in range(4)
    ]
    brd = [
        nc.declare_dram_parameter(f"br{l}", [P, NKD], FP32, isOutput=False)
        for l in range(4)
    ]
    bmask_d = nc.declare_dram_parameter("bmask", [P, NTOT], FP32, isOutput=False)
    selcol_d = nc.declare_dram_parameter("selcol", [P, NSLOT], FP32, isOutput=False)
    iotac_d = nc.declare_dram_parameter("iotac", [P, CCH], FP32, isOutput=False)
    out_d = nc.declare_dram_parameter("out", [NSLOT, D], FP32, isOutput=True)

    with tile.TileContext(nc) as tc, ExitStack() as ctx:
        sb = ctx.enter_context(tc.tile_pool(name="sb", bufs=1))
        big = ctx.enter_context(tc.tile_pool(name="big", bufs=NKI + 2 * NKD))
        wp = ctx.enter_context(tc.tile_pool(name="wp", bufs=1))
        tp = ctx.enter_context(tc.tile_pool(name="tp", bufs=2))
        pp = ctx.enter_context(tc.tile_pool(name="pp", bufs=8, space="PSUM"))

        # ---- constants ----
        identity = sb.tile([P, P], FP32, tag="identity")
        make_identity(nc, identity[:])
        bmask = sb.tile([P, NTOT], FP32, tag="bmask")
        nc.sync.dma_start(out=bmask[:], in_=bmask_d[:, :])
        selcol = sb.tile([P, NSLOT], FP32, tag="selcol")
        nc.sync.dma_start(out=selcol[:], in_=selcol_d[:, :])
        iotac = sb.tile([P, CCH], FP32, tag="iotac")
        nc.sync.dma_start(out=iotac[:], in_=iotac_d[:, :])
        bfs, brs = [], []
        for l in range(4):
            bft = sb.tile([P, NKD], FP32, tag=f"bf{l}")
            nc.sync.dma_start(out=bft[:], in_=bfd[l][:, :])
            bfs.append(bft)
            brt = sb.tile([P, NKD], FP32, tag=f"br{l}")
            nc.sync.dma_start(out=brt[:], in_=brd[l][:, :])
            brs.append(brt)
        idx_sb = sb.tile([P, NJ], I32, tag="idx")
        nc.sync.dma_start(
            out=idx_sb[:], in_=xidx[:].rearrange("(j p) -> p j", p=P)
        )

        # ---- embedding gather + transpose to channel-major e_T ----
        eT = []
        for k in range(NKI):
            et = big.tile([P, NTOT], BF16, tag="hb", name=f"eT{k}")
            eT.append(et)
        for j, (ts_, tj) in enumerate(TOK_T):
            eg = sb.tile([P, K], FP32, tag="eg", bufs=2, name=f"eg{j}")
            nc.gpsimd.indirect_dma_start(
                out=eg[:tj, :],
                out_offset=None,
                in_=emb[:, :],
                in_offset=bass.IndirectOffsetOnAxis(
                    ap=idx_sb[:tj, j : j + 1], axis=0
                ),
            )
            for k, (ks, kk) in enumerate(KT_IN):
                pt = pp.tile([P, PSW], FP32, tag="pt", name=f"ptr{j}_{k}")
                nc.tensor.transpose(
                    out=pt[:kk, :tj],
                    in_=eg[:tj, ks : ks + kk],
                    identity=identity[:tj, :tj],
                )
                nc.vector.tensor_copy(
                    out=eT[k][:kk, ts_ : ts_ + tj], in_=pt[:kk, :tj]
                )

        # ---- SRU layers ----
        hsel = []
        for ci in range(NKD):
            hs = sb.tile([P, NSLOT], FP32, tag="hsel", bufs=NKD, name=f"hsel{ci}")
            hsel.append(hs)

        in_tiles = eT
        ktin = KT_IN
        for l in range(4):
            gates = 4 if l == 0 else 3
            Kin = K if l == 0 else D
            nki = len(ktin)
            main_rows = (nki - 1) * P
            rem = Kin - main_rows

            new_h = None
            if l < 3:
                new_h = [
                    big.tile([P, NTOT], BF16, tag="hb", name=f"h{l}_{ci}")
                    for ci in range(NKD)
                ]

            for ci, (cs, mc) in enumerate(KT_D):
                wts = []
                for g in range(gates):
                    wt = wp.tile(
                        [P, nki, P],
                        BF16,
                        tag=f"w{g}",
                        bufs=2,
                        name=f"w{l}_{ci}_{g}",
                    )
                    if W_CONTIG:
                        nc.sync.dma_start(
                            out=wt[:, :, :],
                            in_=Wd[l][ci * gates + g, :, :].rearrange(
                                "p (kt m) -> p kt m", m=P
                            ),
                        )
                    else:
                        col0 = g * D + cs
                        if main_rows > 0:
                            nc.sync.dma_start(
                                out=wt[:, : nki - 1, :mc],
                                in_=Wd[l][0:main_rows, col0 : col0 + mc].rearrange(
                                    "(kt p) n -> p kt n", p=P
                                ),
                            )
                        nc.sync.dma_start(
                            out=wt[:rem, nki - 1, :mc],
                            in_=Wd[l][main_rows:Kin, col0 : col0 + mc],
                        )
                    wts.append(wt)

                for n in range(NCH):
                    nsl = slice(n * CCH, (n + 1) * CCH)
                    ps = []
                    for g in range(gates):
                        pt = pp.tile(
                            [P, PSW], FP32, tag="pt", name=f"pm{l}_{ci}_{n}_{g}"
                        )
                        for k2, (ks2, kk2) in enumerate(ktin):
                            nc.tensor.matmul(
                                out=pt[:mc, :CCH],
                                lhsT=wts[g][:kk2, k2, :mc],
                                rhs=in_tiles[k2][:kk2, nsl],
                                start=(k2 == 0),
                                stop=(k2 == nki - 1),
                            )
                        ps.append(pt)
                    cand = ps[0][:mc, :CCH]
                    fpre = ps[1][:mc, :CCH]
                    rpre = ps[2][:mc, :CCH]

                    fsb = tp.tile([P, CCH], FP32, tag="fsb", name=f"f{l}_{ci}_{n}")
                    nc.scalar.activation(
                        out=fsb[:mc, :],
                        in_=fpre,
                        func=AF.Sigmoid,
                        bias=bfs[l][:mc, ci : ci + 1],
                    )
                    rsb = tp.tile([P, CCH], FP32, tag="rsb", name=f"r{l}_{ci}_{n}")
                    nc.scalar.activation(
                        out=rsb[:mc, :],
                        in_=rpre,
                        func=AF.Sigmoid,
                        bias=brs[l][:mc, ci : ci + 1],
                    )
                    # z' = (f - 1) * cand  == -(1-f)*cand
                    zb = tp.tile([P, CCH], FP32, tag="zb", name=f"z{l}_{ci}_{n}")
                    nc.vector.scalar_tensor_tensor(
                        out=zb[:mc, :],
                        in0=fsb[:mc, :],
                        scalar=1.0,
                        in1=cand,
                        op0=OP.subtract,
                        op1=OP.mult,
                    )
                    # f masked at sequence starts (in place)
                    nc.vector.tensor_mul(
                        out=fsb[:mc, :], in0=fsb[:mc, :], in1=bmask[:mc, nsl]
                    )
                    # c_t = fm*c_{t-1} - z'
                    cst = tp.tile([P, CCH], FP32, tag="cst", name=f"c{l}_{ci}_{n}")
                    nc.vector.tensor_tensor_scan(
                        out=cst[:mc, :],
                        data0=fsb[:mc, :],
                        data1=zb[:mc, :],
                        initial=0.0,
                        op0=OP.mult,
                        op1=OP.subtract,
                    )
                    # h = r*tanh(c) + (1-r)*xres, built in place over cst
                    nc.scalar.activation(
                        out=cst[:mc, :], in_=cst[:mc, :], func=AF.Tanh
                    )
                    if l == 0:
                        xres = ps[3][:mc, :CCH]
                    else:
                        xres = in_tiles[ci][:mc, nsl]
                    nc.vector.tensor_sub(out=cst[:mc, :], in0=cst[:mc, :], in1=xres)
                    nc.vector.tensor_mul(
                        out=cst[:mc, :], in0=cst[:mc, :], in1=rsb[:mc, :]
                    )
                    if l < 3:
                        nc.vector.tensor_add(
                            out=new_h[ci][:mc, nsl], in0=cst[:mc, :], in1=xres
                        )
                    else:
                        hh = tp.tile([P, CCH], FP32, tag="hh", name=f"hh{l}_{ci}_{n}")
                        nc.vector.tensor_add(out=hh[:mc, :], in0=cst[:mc, :], in1=xres)
                        # select last-token column per slot:
                        # (iota == selcol[slot]) * h, summed along free
                        for kslot in range(SMAX):
                            slot = n * SMAX + kslot
                            scr = tp.tile(
                                [P, CCH], FP32, tag="scr", name=f"sc{ci}_{n}_{kslot}"
                            )
                            nc.vector.scalar_tensor_tensor(
                                out=scr[:mc, :],
                                in0=iotac[:mc, :],
                                scalar=selcol[:mc, slot : slot + 1],
                                in1=hh[:mc, :],
                                op0=OP.is_equal,
                                op1=OP.mult,
                                accum_out=hsel[ci][:mc, slot : slot + 1],
                            )

            if l < 3:
                in_tiles = new_h
                ktin = KT_D

        # ---- epilogue: transpose selected hidden, L2-normalize, write out ----
        out_sb = sb.tile([NSLOT, D], FP32, tag="out_sb")
        for ci, (cs, mc) in enumerate(KT_D):
            pt = pp.tile([P, PSW], FP32, tag="pt", name=f"pte{ci}")
            nc.tensor.transpose(
                out=pt[:NSLOT, :mc],
                in_=hsel[ci][:mc, :NSLOT],
                identity=identity[:mc, :mc],
            )
            nc.vector.tensor_copy(out=out_sb[:, cs : cs + mc], in_=pt[:NSLOT, :mc])

        # sum of squares, chunked through one PSUM bank
        nq = (D + CCH - 1) // CCH
        ssp = sb.tile([NSLOT, nq], FP32, tag="ssp")
        for q in range(nq):
            q0 = q * CCH
            qw = min(CCH, D - q0)
            pt = pp.tile([P, PSW], FP32, tag="pt", name=f"ptq{q}")
            nc.scalar.activation(
                out=pt[:NSLOT, :qw],
                in_=out_sb[:, q0 : q0 + qw],
                func=AF.Square,
                accum_out=ssp[:, q : q + 1],
            )
        ss = sb.tile([NSLOT, 1], FP32, tag="ss")
        nc.vector.tensor_reduce(
            out=ss[:], in_=ssp[:], axis=mybir.AxisListType.X, op=OP.add
        )
        # 1/sqrt(ss + eps); eps guards empty slots (zero rows)
        eps = sb.tile([NSLOT, 1], FP32, tag="eps")
        nc.gpsimd.memset(eps[:], 1e-20)
        sq = sb.tile([NSLOT, 1], FP32, tag="sq")
        nc.scalar.activation(out=sq[:], in_=ss[:], func=AF.Sqrt, bias=eps[:, 0:1])
        inv = sb.tile([NSLOT, 1], FP32, tag="inv")
        nc.vector.reciprocal(out=inv[:], in_=sq[:])
        nc.vector.tensor_scalar(
            out=out_sb[:], in0=out_sb[:], scalar1=inv[:, 0:1], scalar2=None, op0=OP.mult
        )
        nc.sync.dma_start(out=out_d[:, :], in_=out_sb[:])

    nc.compile()
    return nc


# ---------------------------------------------------------------------------
# Host-side input prep
# ---------------------------------------------------------------------------


def _retile_W(W, Kin, D, gates, NKD, NKI_l):
    """[Kin, gates*D] -> [NKD*gates, 128, nki*128] per-partition-contiguous."""
    import ml_dtypes

    nki = NKI_l
    Wp = np.zeros((nki * P, gates * NKD * P), np.float32)
    Dp = NKD * P
    src = np.asarray(W, np.float32)
    for g in range(gates):
        Wp[:Kin, g * Dp : g * Dp + D] = src[:, g * D : (g + 1) * D]
    # [kt*P+p, g*Dp + ci*P + m] -> [ci*gates+g, p, kt*P+m]
    Wp = Wp.reshape(nki, P, gates, NKD, P)
    Wt = np.ascontiguousarray(
        np.transpose(Wp, (3, 2, 1, 0, 4)).astype(ml_dtypes.bfloat16)
    )
    return Wt.reshape(NKD * gates, P, nki * P)


def _pack_bias(b_half, D, NKD):
    """[D] -> [128, NKD]: column ci holds channels ci*128..ci*128+127."""
    pad = NKD * P - D
    bp = np.pad(np.asarray(b_half, np.float32), (0, pad))
    return np.ascontiguousarray(bp.reshape(NKD, P).T)


def make_core_inputs(core, plan, x, lengths, embed, Ws, bs, K, D, V):
    NCH, CCH, SMAX = plan.NCH, plan.CCH, plan.SMAX
    NTOT, NSLOT = plan.NTOT, plan.NSLOT
    NTOKP = ((NTOT + P - 1) // P) * P
    NKD = len(_ptiles(D))

    xl = np.zeros(NTOKP, np.int32)
    bmask = np.zeros((1, NTOT), np.float32)
    selcol = np.full((1, NSLOT), -1.0, np.float32)
    for ch, bin_seqs in enumerate(plan.bins[core]):
        pos = 0
        for k, b in enumerate(bin_seqs):
            ln = int(lengths[b])
            if ln <= 0:
                continue
            col0 = ch * CCH + pos
            xl[col0 : col0 + ln] = x[b, :ln]
            bmask[0, col0 + 1 : col0 + ln] = 1.0
            selcol[0, ch * SMAX + k] = float(pos + ln - 1)
            pos += ln

    iotac = np.arange(CCH, dtype=np.float32)[None, :]

    im = {
        "xidx": xl,
        "embed": np.asarray(embed, np.float32),
        "bmask": np.broadcast_to(bmask, (P, NTOT)).copy(),
        "selcol": np.broadcast_to(selcol, (P, NSLOT)).copy(),
        "iotac": np.broadcast_to(iotac, (P, CCH)).copy(),
    }
    for l in range(4):
        im[f"W{l}t"] = Ws[l]
        im[f"bf{l}"] = _pack_bias(bs[l][:D], D, NKD)
        im[f"br{l}"] = _pack_bias(bs[l][D:], D, NKD)
    return im


_NC_CACHE = {}


def kernel(x, lengths, embed, W0, b0, W1, b1, W2, b2, W3, b3):
    x = np.asarray(x)
    lengths = np.asarray(lengths)
    embed = np.asarray(embed, np.float32)
    Ws = [np.asarray(w, np.float32) for w in (W0, W1, W2, W3)]
    bs = [np.asarray(b, np.float32) for b in (b0, b1, b2, b3)]

    Bb, T = x.shape
    V, K = embed.shape
    D = Ws[1].shape[0]

    plan = make_plan(lengths)
    key = (K, D, V, plan.NCH, plan.CCH, plan.SMAX)
    if key not in _NC_CACHE:
        _NC_CACHE[key] = build_program(*key)
    nc = _NC_CACHE[key]

    NKD = len(_ptiles(D))
    NKI = len(_ptiles(K))
    Wt = [
        _retile_W(Ws[0], K, D, 4, NKD, NKI),
        _retile_W(Ws[1], D, D, 3, NKD, NKD),
        _retile_W(Ws[2], D, D, 3, NKD, NKD),
        _retile_W(Ws[3], D, D, 3, NKD, NKD),
    ]
    in_maps = [
        make_core_inputs(c, plan, x, lengths, embed, Wt, bs, K, D, V)
        for c in range(N_CORES)
    ]
    res = run_bass_kernel_spmd(nc, in_maps, core_ids=list(range(N_CORES)))

    out = np.zeros((Bb, D), np.float32)
    for c in range(N_CORES):
        oc = res.results[c]["out"]
        for ch, bin_seqs in enumerate(plan.bins[c]):
            for k, b in enumerate(bin_seqs):
                out[b] = oc[ch * plan.SMAX + k]
    return out



# revision 8
# speedup vs baseline: 18.2343x; 7.5721x over previous
"""Trainium2 Bass kernel for nn_EncoderText (4-layer SRU text encoder).

Reference computation:
  e = embed[x]                       # [B, T, K]
  4x SRU layers over time (layer0: k=4 projections incl highway; 1-3: k=3)
  gather last valid timestep per sequence, L2-normalize over features.

Strategy:
- Data-parallel over sequences across 8 NeuronCores, no collectives.
- Only VALID tokens are computed: sequences are LPT-bin-packed into
  NCH chunks of CCH columns per core (all cores share one program
  shape; the assignment of sequences to cores/chunks is host-side).
- Activations are channel-major [D, tokens] so the SRU recurrence
  c_t = f_t*c_{t-1} + (1-f_t)*cand_t maps onto the DVE
  tensor_tensor_scan along the free axis; a boundary mask (runtime
  input) zeroes f at each sequence start so sequences chain safely
  through one scan per chunk.
- Matmuls run in bf16 (guaranteed 1 cycle/row on the PE; fp32r is
  quarter-rate on real HW for these shapes). PSUM accumulates fp32.
- DVE recurrence tiles are fp16 (2x DVE mode where all operands are
  2-byte); the scan itself stays accurate enough (10-bit mantissa).
- Last-timestep selection: transpose h3 chunks on the PE, then a
  one-hot matmul gathers each slot's last-token column straight into
  [slot, channel] orientation (no DVE is_equal pass, no epilogue
  transpose); slot->sequence mapping is undone on the host.
"""

from contextlib import ExitStack

import numpy as np

import concourse.bass as bass
import concourse.mybir as mybir
import concourse.tile as tile
from concourse import bacc
from concourse.bass_utils import run_bass_kernel_spmd
from concourse.masks import make_identity

FP32 = mybir.dt.float32
FP16 = mybir.dt.float16
BF16 = mybir.dt.bfloat16
I32 = mybir.dt.int32
AF = mybir.ActivationFunctionType
OP = mybir.AluOpType

P = 128
N_CORES = 8


def _ptiles(n):
    """[(start, size)] partition tiles of <=128 covering n."""
    out = []
    s = 0
    while s < n:
        out.append((s, min(P, n - s)))
        s += P
    return out


# ---------------------------------------------------------------------------
# Host-side packing plan
# ---------------------------------------------------------------------------


class Plan:
    def __init__(self, NCH, CCH, SMAX, bins):
        self.NCH = NCH  # chunks per core
        self.CCH = CCH  # columns per chunk
        self.SMAX = SMAX  # max sequences per chunk
        self.bins = bins  # [N_CORES][NCH] -> list of global seq ids
        self.NTOT = NCH * CCH
        self.NSLOT = NCH * SMAX


def make_plan(lengths):
    """LPT bin-pack sequences into N_CORES*NCH bins of capacity CCH."""
    lengths = np.asarray(lengths, np.int64)
    order = np.argsort(-lengths, kind="stable")
    maxlen = int(lengths.max())

    best = None
    for NCH in range(1, 17):
        nbins = N_CORES * NCH
        loads = np.zeros(nbins, np.int64)
        bins = [[] for _ in range(nbins)]
        for i in order:
            j = int(np.argmin(loads))
            loads[j] += int(lengths[i])
            bins[j].append(int(i))
        C = int(loads.max())
        CCH = max(256, maxlen, -(-C // 8) * 8)
        if CCH > 512:
            continue
        cost = NCH * CCH
        if best is None or cost < best[0]:
            SMAX = max(len(b) for b in bins)
            best = (cost, NCH, CCH, SMAX, bins)
        if NCH * 256 >= best[0]:
            break
    assert best is not None, "no feasible packing"
    _, NCH, CCH, SMAX, bins = best
    core_bins = [bins[c * NCH : (c + 1) * NCH] for c in range(N_CORES)]
    plan = Plan(NCH, CCH, SMAX, core_bins)
    assert plan.NSLOT <= P, f"too many slots {plan.NSLOT}"
    return plan


# ---------------------------------------------------------------------------
# Device program
# ---------------------------------------------------------------------------


def build_program(K, D, V, NCH, CCH, SMAX):
    """Emit the per-core program.

    DRAM parameters (per core):
      xidx   [NTOKP]      int32  packed token row indices (pad -> 0)
      embed  [V, K]       f32    full embedding table (replicated)
      W0 [K, 4D], W1..W3 [D, 3D] bf16, host-pre-tiled
      bf{l}, br{l} [128, NKD] f32  per-channel-tile bias columns
      bmask  [128, NTOT]  f16    0 at each sequence start (and padding), else 1
      onehot [128, NCH*JC*SMAX] bf16  one-hot of each slot's last column
      out    [NSLOT, D]   f32    normalized last-step hidden per slot
    """
    NTOT = NCH * CCH
    NSLOT = NCH * SMAX
    NTOKP = ((NTOT + P - 1) // P) * P
    NJ = NTOKP // P
    KT_IN = _ptiles(K)
    KT_D = _ptiles(D)
    NKI, NKD = len(KT_IN), len(KT_D)
    TOK_T = _ptiles(NTOT)
    JT = _ptiles(CCH)  # column chunks of a CCH chunk (for selection)
    JC = len(JT)
    PSW = max(CCH, P)

    nc = bacc.Bacc("TRN2", target_bir_lowering=False, debug=False)

    xidx = nc.declare_dram_parameter("xidx", [NTOKP], I32, isOutput=False)
    emb = nc.declare_dram_parameter("embed", [V, K], FP32, isOutput=False)
    Wd = []
    for l in range(4):
        gates_l = 4 if l == 0 else 3
        nki_l = len(KT_IN) if l == 0 else len(KT_D)
        Wd.append(
            nc.declare_dram_parameter(
                f"W{l}t", [NKD * gates_l, P, nki_l * P], BF16, isOutput=False
            )
        )
    bfd = [
        nc.declare_dram_parameter(f"bf{l}", [P, NKD], FP32, isOutput=False)
        for l in range(4)
    ]
    brd = [
        nc.declare_dram_parameter(f"br{l}", [P, NKD], FP32, isOutput=False)
        for l in range(4)
    ]
    bmask_d = nc.declare_dram_parameter("bmask", [P, NTOT], FP16, isOutput=False)
    onehot_d = nc.declare_dram_parameter(
        "onehot", [P, NCH * JC * SMAX], BF16, isOutput=False
    )
    out_d = nc.declare_dram_parameter("out", [NSLOT, D], FP32, isOutput=True)

    with tile.TileContext(nc) as tc, ExitStack() as ctx:
        sb = ctx.enter_context(tc.tile_pool(name="sb", bufs=1))
        big = ctx.enter_context(tc.tile_pool(name="big", bufs=NKI + 2 * NKD))
        wp = ctx.enter_context(tc.tile_pool(name="wp", bufs=1))
        tp = ctx.enter_context(tc.tile_pool(name="tp", bufs=2))
        pp = ctx.enter_context(tc.tile_pool(name="pp", bufs=5, space="PSUM"))
        sp = ctx.enter_context(tc.tile_pool(name="sp", bufs=1, space="PSUM"))

        # ---- constants ----
        identity = sb.tile([P, P], FP32, tag="identity")
        make_identity(nc, identity[:])
        identity_bf = sb.tile([P, P], BF16, tag="identity_bf")
        nc.vector.tensor_copy(out=identity_bf[:], in_=identity[:])
        bmask = sb.tile([P, NTOT], FP16, tag="bmask")
        nc.sync.dma_start(out=bmask[:], in_=bmask_d[:, :])
        onehot = sb.tile([P, NCH * JC * SMAX], BF16, tag="onehot")
        nc.sync.dma_start(out=onehot[:], in_=onehot_d[:, :])
        bfs, brs = [], []
        for l in range(4):
            bft = sb.tile([P, NKD], FP32, tag=f"bf{l}")
            nc.sync.dma_start(out=bft[:], in_=bfd[l][:, :])
            bfs.append(bft)
            brt = sb.tile([P, NKD], FP32, tag=f"br{l}")
            nc.sync.dma_start(out=brt[:], in_=brd[l][:, :])
            brs.append(brt)
        idx_sb = sb.tile([P, NJ], I32, tag="idx")
        nc.sync.dma_start(
            out=idx_sb[:], in_=xidx[:].rearrange("(j p) -> p j", p=P)
        )

        # ---- embedding gather + transpose to channel-major e_T ----
        eT = []
        for k in range(NKI):
            et = big.tile([P, NTOT], BF16, tag="hb", name=f"eT{k}")
            eT.append(et)
        for j, (ts_, tj) in enumerate(TOK_T):
            eg = sb.tile([P, K], FP32, tag="eg", bufs=2, name=f"eg{j}")
            nc.gpsimd.indirect_dma_start(
                out=eg[:tj, :],
                out_offset=None,
                in_=emb[:, :],
                in_offset=bass.IndirectOffsetOnAxis(
                    ap=idx_sb[:tj, j : j + 1], axis=0
                ),
            )
            for k, (ks, kk) in enumerate(KT_IN):
                pt = pp.tile([P, PSW], FP32, tag="pt", name=f"ptr{j}_{k}")
                nc.tensor.transpose(
                    out=pt[:kk, :tj],
                    in_=eg[:tj, ks : ks + kk],
                    identity=identity[:tj, :tj],
                )
                nc.vector.tensor_copy(
                    out=eT[k][:kk, ts_ : ts_ + tj], in_=pt[:kk, :tj]
                )

        # ---- SRU layers ----
        out_sb = sb.tile([NSLOT, D], FP32, tag="out_sb")

        in_tiles = eT
        ktin = KT_IN
        for l in range(4):
            gates = 4 if l == 0 else 3
            nki = len(ktin)

            new_h = None
            if l < 3:
                new_h = [
                    big.tile([P, NTOT], BF16, tag="hb", name=f"h{l}_{ci}")
                    for ci in range(NKD)
                ]

            for ci, (cs, mc) in enumerate(KT_D):
                wts = []
                for g in range(gates):
                    wt = wp.tile(
                        [P, nki, P],
                        BF16,
                        tag=f"w{g}",
                        bufs=2,
                        name=f"w{l}_{ci}_{g}",
                    )
                    nc.sync.dma_start(
                        out=wt[:, :, :],
                        in_=Wd[l][ci * gates + g, :, :].rearrange(
                            "p (kt m) -> p kt m", m=P
                        ),
                    )
                    wts.append(wt)

                if l == 3:
                    hsel_ps = sp.tile(
                        [P, P], FP32, tag="hsel", bufs=1, name=f"hsel{ci}"
                    )

                for n in range(NCH):
                    nsl = slice(n * CCH, (n + 1) * CCH)
                    ps = []
                    for g in range(gates):
                        pt = pp.tile(
                            [P, PSW], FP32, tag="pt", name=f"pm{l}_{ci}_{n}_{g}"
                        )
                        for k2, (ks2, kk2) in enumerate(ktin):
                            nc.tensor.matmul(
                                out=pt[:mc, :CCH],
                                lhsT=wts[g][:kk2, k2, :mc],
                                rhs=in_tiles[k2][:kk2, nsl],
                                start=(k2 == 0),
                                stop=(k2 == nki - 1),
                            )
                        ps.append(pt)
                    cand = ps[0][:mc, :CCH]
                    fpre = ps[1][:mc, :CCH]
                    rpre = ps[2][:mc, :CCH]

                    fsb = tp.tile([P, CCH], FP16, tag="fsb", name=f"f{l}_{ci}_{n}")
                    nc.scalar.activation(
                        out=fsb[:mc, :],
                        in_=fpre,
                        func=AF.Sigmoid,
                        bias=bfs[l][:mc, ci : ci + 1],
                    )
                    rsb = tp.tile([P, CCH], FP16, tag="rsb", name=f"r{l}_{ci}_{n}")
                    nc.scalar.activation(
                        out=rsb[:mc, :],
                        in_=rpre,
                        func=AF.Sigmoid,
                        bias=brs[l][:mc, ci : ci + 1],
                    )
                    # z' = (f - 1) * cand  == -(1-f)*cand
                    zb = tp.tile([P, CCH], FP16, tag="zb", name=f"z{l}_{ci}_{n}")
                    nc.vector.scalar_tensor_tensor(
                        out=zb[:mc, :],
                        in0=fsb[:mc, :],
                        scalar=1.0,
                        in1=cand,
                        op0=OP.subtract,
                        op1=OP.mult,
                    )
                    # f masked at sequence starts (in place)
                    nc.vector.tensor_mul(
                        out=fsb[:mc, :], in0=fsb[:mc, :], in1=bmask[:mc, nsl]
                    )
                    # c_t = fm*c_{t-1} - z'
                    cst = tp.tile([P, CCH], FP16, tag="cst", name=f"c{l}_{ci}_{n}")
                    nc.vector.tensor_tensor_scan(
                        out=cst[:mc, :],
                        data0=fsb[:mc, :],
                        data1=zb[:mc, :],
                        initial=0.0,
                        op0=OP.mult,
                        op1=OP.subtract,
                    )
                    # h = r*tanh(c) + (1-r)*xres, built in place over cst
                    nc.scalar.activation(
                        out=cst[:mc, :], in_=cst[:mc, :], func=AF.Tanh
                    )
                    if l == 0:
                        xres = ps[3][:mc, :CCH]
                    else:
                        xres = in_tiles[ci][:mc, nsl]
                    nc.vector.tensor_sub(out=cst[:mc, :], in0=cst[:mc, :], in1=xres)
                    nc.vector.tensor_mul(
                        out=cst[:mc, :], in0=cst[:mc, :], in1=rsb[:mc, :]
                    )
                    if l < 3:
                        nc.vector.tensor_add(
                            out=new_h[ci][:mc, nsl], in0=cst[:mc, :], in1=xres
                        )
                    else:
                        hh = tp.tile([P, CCH], BF16, tag="hh", name=f"hh{l}_{ci}_{n}")
                        nc.vector.tensor_add(out=hh[:mc, :], in0=cst[:mc, :], in1=xres)
                        # transpose h chunk on PE, then one-hot matmul
                        # gathers each slot's last column: [slot, channel]
                        htp = sp.tile(
                            [P, JC, P], BF16, tag="htp", bufs=2, name=f"htp{ci}_{n}"
                        )
                        hts = tp.tile(
                            [P, JC, P], BF16, tag="hts", name=f"hts{ci}_{n}"
                        )
                        for jj, (js, jw) in enumerate(JT):
                            nc.tensor.transpose(
                                out=htp[:jw, jj, :mc],
                                in_=hh[:mc, js : js + jw],
                                identity=identity_bf[:mc, :mc],
                            )
                            nc.vector.tensor_copy(
                                out=hts[:jw, jj, :mc], in_=htp[:jw, jj, :mc]
                            )
                        for jj, (js, jw) in enumerate(JT):
                            nc.tensor.matmul(
                                out=hsel_ps[n * SMAX : (n + 1) * SMAX, :mc],
                                lhsT=onehot[
                                    :jw, (n * JC + jj) * SMAX : (n * JC + jj + 1) * SMAX
                                ],
                                rhs=hts[:jw, jj, :mc],
                                start=(jj == 0),
                                stop=(jj == JC - 1),
                            )
                if l == 3:
                    nc.vector.tensor_copy(
                        out=out_sb[:, cs : cs + mc], in_=hsel_ps[:NSLOT, :mc]
                    )

            if l < 3:
                in_tiles = new_h
                ktin = KT_D

        # ---- epilogue: L2-normalize rows of out_sb, write out ----
        nq = (D + CCH - 1) // CCH
        ssp = sb.tile([NSLOT, nq], FP32, tag="ssp")
        for q in range(nq):
            q0 = q * CCH
            qw = min(CCH, D - q0)
            pt = pp.tile([P, PSW], FP32, tag="pt", name=f"ptq{q}")
            nc.scalar.activation(
                out=pt[:NSLOT, :qw],
                in_=out_sb[:, q0 : q0 + qw],
                func=AF.Square,
                accum_out=ssp[:, q : q + 1],
            )
        ss = sb.tile([NSLOT, 1], FP32, tag="ss")
        nc.vector.tensor_reduce(
            out=ss[:], in_=ssp[:], axis=mybir.AxisListType.X, op=OP.add
        )
        # 1/sqrt(ss + eps); eps guards empty slots (zero rows)
        eps = sb.tile([NSLOT, 1], FP32, tag="eps")
        nc.gpsimd.memset(eps[:], 1e-20)
        sq = sb.tile([NSLOT, 1], FP32, tag="sq")
        nc.scalar.activation(out=sq[:], in_=ss[:], func=AF.Sqrt, bias=eps[:, 0:1])
        inv = sb.tile([NSLOT, 1], FP32, tag="inv")
        nc.vector.reciprocal(out=inv[:], in_=sq[:])
        nc.vector.tensor_scalar(
            out=out_sb[:], in0=out_sb[:], scalar1=inv[:, 0:1], scalar2=None, op0=OP.mult
        )
        nc.sync.dma_start(out=out_d[:, :], in_=out_sb[:])

    nc.compile()
    return nc


# ---------------------------------------------------------------------------
# Host-side input prep
# ---------------------------------------------------------------------------


def _retile_W(W, Kin, D, gates, NKD, NKI_l):
    """[Kin, gates*D] -> [NKD*gates, 128, nki*128] per-partition-contiguous."""
    import ml_dtypes

    nki = NKI_l
    Wp = np.zeros((nki * P, gates * NKD * P), np.float32)
    Dp = NKD * P
    src = np.asarray(W, np.float32)
    for g in range(gates):
        Wp[:Kin, g * Dp : g * Dp + D] = src[:, g * D : (g + 1) * D]
    # [kt*P+p, g*Dp + ci*P + m] -> [ci*gates+g, p, kt*P+m]
    Wp = Wp.reshape(nki, P, gates, NKD, P)
    Wt = np.ascontiguousarray(
        np.transpose(Wp, (3, 2, 1, 0, 4)).astype(ml_dtypes.bfloat16)
    )
    return Wt.reshape(NKD * gates, P, nki * P)


def _pack_bias(b_half, D, NKD):
    """[D] -> [128, NKD]: column ci holds channels ci*128..ci*128+127."""
    pad = NKD * P - D
    bp = np.pad(np.asarray(b_half, np.float32), (0, pad))
    return np.ascontiguousarray(bp.reshape(NKD, P).T)


def make_core_inputs(core, plan, x, lengths, embed, Ws, bs, K, D, V):
    import ml_dtypes

    NCH, CCH, SMAX = plan.NCH, plan.CCH, plan.SMAX
    NTOT, NSLOT = plan.NTOT, plan.NSLOT
    NTOKP = ((NTOT + P - 1) // P) * P
    NKD = len(_ptiles(D))
    JC = (CCH + P - 1) // P

    xl = np.zeros(NTOKP, np.int32)
    bmask = np.zeros((1, NTOT), np.float16)
    onehot = np.zeros((P, NCH * JC * SMAX), np.float32)
    for ch, bin_seqs in enumerate(plan.bins[core]):
        pos = 0
        for k, b in enumerate(bin_seqs):
            ln = int(lengths[b])
            if ln <= 0:
                continue
            col0 = ch * CCH + pos
            xl[col0 : col0 + ln] = x[b, :ln]
            bmask[0, col0 + 1 : col0 + ln] = 1.0
            pl = pos + ln - 1  # in-chunk column of last token
            onehot[pl % P, (ch * JC + pl // P) * SMAX + k] = 1.0
            pos += ln

    im = {
        "xidx": xl,
        "embed": np.asarray(embed, np.float32),
        "bmask": np.broadcast_to(bmask, (P, NTOT)).copy(),
        "onehot": onehot.astype(ml_dtypes.bfloat16),
    }
    for l in range(4):
        im[f"W{l}t"] = Ws[l]
        im[f"bf{l}"] = _pack_bias(bs[l][:D], D, NKD)
        im[f"br{l}"] = _pack_bias(bs[l][D:], D, NKD)
    return im


_NC_CACHE = {}


def kernel(x, lengths, embed, W0, b0, W1, b1, W2, b2, W3, b3):
    x = np.asarray(x)
    lengths = np.asarray(lengths)
    embed = np.asarray(embed, np.float32)
    Ws = [np.asarray(w, np.float32) for w in (W0, W1, W2, W3)]
    bs = [np.asarray(b, np.float32) for b in (b0, b1, b2, b3)]

    Bb, T = x.shape
    V, K = embed.shape
    D = Ws[1].shape[0]

    plan = make_plan(lengths)
    key = (K, D, V, plan.NCH, plan.CCH, plan.SMAX)
    if key not in _NC_CACHE:
        _NC_CACHE[key] = build_program(*key)
    nc = _NC_CACHE[key]

    NKD = len(_ptiles(D))
    NKI = len(_ptiles(K))
    Wt = [
        _retile_W(Ws[0], K, D, 4, NKD, NKI),
        _retile_W(Ws[1], D, D, 3, NKD, NKD),
        _retile_W(Ws[2], D, D, 3, NKD, NKD),
        _retile_W(Ws[3], D, D, 3, NKD, NKD),
    ]
    in_maps = [
        make_core_inputs(c, plan, x, lengths, embed, Wt, bs, K, D, V)
        for c in range(N_CORES)
    ]
    res = run_bass_kernel_spmd(nc, in_maps, core_ids=list(range(N_CORES)))

    out = np.zeros((Bb, D), np.float32)
    for c in range(N_CORES):
        oc = res.results[c]["out"]
        for ch, bin_seqs in enumerate(plan.bins[c]):
            for k, b in enumerate(bin_seqs):
                out[b] = oc[ch * plan.SMAX + k]
    return out
